# revision 38
# baseline (speedup 1.0000x reference)
"""Trainium2 Bass kernel for nn_AttentionLayer (sparse_attention).

Reference computation (per batch b):
    q = wq @ x + bq          [8, N]     (1x1 conv, d=8, N=H*W=4096)
    k = wk @ x + bk          [8, N]
    v = wv @ x + bv          [64, N]
    energy = q^T k           [N, N]
    attn = softmax(energy, axis=-1)
    out = gamma * (v @ attn^T) + x
Sharding: data-parallel over batch; one batch element per NeuronCore.

Device-side work (the measured NEFF): q/k/v projections, the N x N
energy matmuls, softmax, the output matmuls, normalize, residual.
Host-side (kernel(), unmeasured like any input sharding): weight
REPACKING only - transposes, SQK/gamma/bias folding, 16x replication,
bf16/f32 dtype staging of x.  No model matmuls happen on host.

Architecture (hardware-measured rates drove every choice):
  - PE psum write port = 128 partitions x 1 column/cycle @ 2.4 GHz is
    the matmul wall: energy emits N^2/128 = 131k columns (55 us), the
    out accumulation 16 slab-passes x 4096 i / 512-per-bank = 65k
    columns (27 us).  Row-tiled / partial-K matmuls share the same port
    (measured) AND de-assert the HAM activity monitor (PE drops to 1.2
    GHz), so energy matmuls stay plain K=128 fp8 (16 replicas of the
    d=8 q/k, SQK-scaled so psum = A5 * q.k exactly).
  - Out (v @ attn^T): DoubleRow fp8: lhsT = vT jb-pair [128, 2, 128],
    rhs = aT [128, 2, 512]; vT col 64 = ones accumulates the softmax
    denominator in psum row 64.
  - exp: split ACT (true exp -> fp8e5, (FD+352)/1.2 ns) and DVE
    (Schraudolph e5m2 bits: i8(round(psum + B5)), (FD+120)/0.96 ns) -
    the ONLY two engines that read PSUM; their combined stream rate
    (2.16 elem/ns) is the softmax floor (~61 us for 128k FD).
  - normalize on Pool (gpsimd, idle otherwise): 1/s via bf16 magic
    seed + one Newton step; r broadcast over partitions via a DRAM
    round-trip; y = x - yu*(-r) with the residual add in f32.
  - prep: every DMA costs ~600ns of queue time; x lands via both HWDGE
    queues; chunk emission interleaves with the first i-pair's units so
    the attention loop starts as soon as chunks 0-1 are evacuated.

Accuracy: fp8 q/k/v + e5m2 attn weights + bf16 normalize cost ~3e-3
final relative error (tolerance 2e-2).
"""

import os
import sys

import numpy as np

sys.path.insert(0, "/opt/trn_rl_repo")

B, C, HH, WW = 8, 64, 64, 64
N = HH * WW  # 4096
D = 8  # qk channels
IC = 512  # i-chunk
N_IC = N // IC  # 8
JB = 128  # j-block
N_JB = N // JB  # 32
NP = N_JB // 2  # 16 jb-pairs

A5 = float(4.0 / np.log(2.0))
B5 = float(4 * 15.0 - 0.5)
SQK = float(0.25 * np.sqrt(A5))  # per-side scale: 16 reps * SQK^2 = A5
K16 = 0x7EF0  # bf16 reciprocal magic
PIPE = 4

# jb's whose exp runs on ACT (17, evenly spread); rest on DVE (15)
ACT_SET = frozenset(j for j in range(32) if (j * 17) // 32 != ((j + 1) * 17) // 32)

_CACHE = {}


def _build_program():
    import concourse.bass as bass
    import concourse.tile as tile
    from concourse import bacc, mybir

    f32 = mybir.dt.float32
    bf16 = mybir.dt.bfloat16
    i8 = mybir.dt.int8
    i16 = mybir.dt.int16
    fp8e4 = mybir.dt.float8e4
    fp8e5 = mybir.dt.float8e5
    EXP = mybir.ActivationFunctionType.Exp
    DR = mybir.MatmulPerfMode.DoubleRow
    MUL = mybir.AluOpType.mult
    ADD = mybir.AluOpType.add
    SUB = mybir.AluOpType.subtract

    nc = bacc.Bacc(
        "TRN2", target_bir_lowering=False, debug=False, enable_asserts=False
    )

    # Host-prepacked inputs (see _host_pack): xb = [x; ones; zeros] bf16,
    # wqt/wkt = replicated SQK-scaled [wq;bq]^T bf16 [128, 128],
    # wvt = [gamma wv^T; gamma bv; zeros] bf16 [128, 64].
    x_d = nc.dram_tensor("x", [C, N], f32, kind="ExternalInput").ap()
    xb_d = nc.dram_tensor("xb", [2 * C, N], bf16, kind="ExternalInput").ap()
    wqt_d = nc.dram_tensor("wqt", [2 * C, 2 * C], bf16, kind="ExternalInput").ap()
    wkt_d = nc.dram_tensor("wkt", [2 * C, 2 * C], bf16, kind="ExternalInput").ap()
    wvt_d = nc.dram_tensor("wvt", [2 * C, C], bf16, kind="ExternalInput").ap()
    y_d = nc.dram_tensor("y", [C, N], f32, kind="ExternalOutput").ap()
    r_d = nc.dram_tensor("r_scr", [5, 2 * IC], bf16, kind="Internal").ap()

    with tile.TileContext(nc) as tc:
        from contextlib import ExitStack

        with ExitStack() as ctx:
            consts = ctx.enter_context(tc.tile_pool(name="consts", bufs=1))
            bigs = ctx.enter_context(tc.tile_pool(name="bigs", bufs=1))
            work = ctx.enter_context(tc.tile_pool(name="work", bufs=4))
            ypool = ctx.enter_context(tc.tile_pool(name="ypool", bufs=2))
            small = ctx.enter_context(tc.tile_pool(name="small", bufs=4))

            # ---------------- DMAs ----------------
            # Weights first (tiny), then xb in 3 pieces split across both
            # HWDGE queues so chunk 0 lands earliest, then xf32 (only
            # needed for the residual from ~35us on).
            wqT = consts.tile([2 * C, 2 * C], bf16)
            wkT = consts.tile([2 * C, 2 * C], bf16)
            wvT2 = consts.tile([2 * C, C], bf16)
            x2c = bigs.tile([2 * C, N], bf16)
            xf32 = bigs.tile([C, N], f32)
            nc.scalar.dma_start(out=x2c[:, 0:IC], in_=xb_d[:, 0:IC])
            nc.scalar.dma_start(out=x2c[:, IC : 2 * IC], in_=xb_d[:, IC : 2 * IC])
            nc.scalar.dma_start(
                out=x2c[:, 2 * IC : 5 * IC], in_=xb_d[:, 2 * IC : 5 * IC]
            )
            nc.sync.dma_start(out=wqT, in_=wqt_d)
            nc.sync.dma_start(out=wkT, in_=wkt_d)
            nc.sync.dma_start(out=wvT2, in_=wvt_d)
            nc.sync.dma_start(out=x2c[:, 5 * IC :], in_=xb_d[:, 5 * IC :])
            nc.sync.dma_start(out=xf32[:, 0 : N // 2], in_=x_d[:, 0 : N // 2])
            nc.sync.dma_start(out=xf32[:, N // 2 :], in_=x_d[:, N // 2 :])

            # warm-up stationary: depends only on a DVE memset
            wconst = consts.tile([C, C], bf16)
            nc.vector.memset(wconst.bitcast(f32), 0.0)
            mones = consts.tile([65, C], bf16)
            nc.vector.memset(mones, -1.0)

            # vT: [128, NP, 2, 128] fp8e4; [j, p, s, 0:64] = gamma*v^T for
            # j-block 2p+s, col 64 = ones (denominator), cols 65:127 = 0.
            # On GPSIMD (idle in prep; no longer fights make_identity).
            vT = bigs.tile([JB, NP, 2, JB], fp8e4)
            nc.gpsimd.memset(vT[:, :, :, C : C + 1], 1.0)
            nc.gpsimd.memset(vT[:, :, :, C + 1 :], 0.0)

            # warm the Exp activation table early (table load ~1.3us)
            warm = consts.tile([1, 8], f32)
            nc.scalar.activation(warm, wconst[0:1, 0:8], EXP)

            # Unified PSUM rotation: warmers, prep projections and loop
            # energy tiles share ONE bufs=3 pool of [128, 1024] f32 tiles
            # (2 banks) -- no pool-close barrier between warm-up and the
            # first projection, and the loop's first energy matmuls chase
            # the prep evacuations through the same rotation.
            psum_e = ctx.enter_context(
                tc.tile_pool(name="psum_e", bufs=3, space="PSUM")
            )
            psum_o = ctx.enter_context(
                tc.tile_pool(name="psum_o", bufs=1, space="PSUM")
            )

            # ramp-warmer: PE busy from ~6.6us so the HAM clock grant hits
            # full speed before/while the projections run.
            wsc = psum_e.tile([JB, 2 * IC], f32, tag="eps", name="wsc")
            for _ in range(8):
                nc.tensor.matmul(
                    wsc[0:C, 0:C], wconst, wconst, start=True, stop=True
                )

            # ---------------- projections ----------------
            # qk8 [128, 2, N] fp8e4: slab 0 = q, slab 1 = k (16 partition-
            # replicas each).  q and k land in ONE psum tile per chunk so a
            # single FD-1024 evacuation copy moves both; copies alternate
            # ACT/DVE.  v psums pair 2 chunks -> one FD-512 ACT copy.
            # Chunk emission interleaves with the first i-pair's units
            # (need_chunks) so the attention loop starts once chunks 0-1
            # are evacuated.
            # bridge warmers: keep the PE busy between the early warm block
            # and the first projection (gated by the x DMA landing ~11.5us)
            # so the HAM activity window never sees an idle gap.  They live
            # in psum_o's banks, whose first real use is much later.
            brg = psum_o.tile([2 * C, 2 * IC], f32, tag="op", name="brg")
            for _ in range(13):
                nc.tensor.matmul(
                    brg[0:C, 0:C], wconst, wconst, start=True, stop=True
                )

            qk8 = bigs.tile([2 * C, 2, N], fp8e4)
            pv2 = [None]
            n_chunks = [0]

            def emit_chunk(ic):
                sl = slice(ic * IC, (ic + 1) * IC)
                xsl = x2c[:, sl]
                pqk = psum_e.tile([2 * C, 2 * IC], f32, tag="eps")
                nc.tensor.matmul(
                    pqk[:, 0:IC], wqT, xsl, start=True, stop=True
                )
                nc.tensor.matmul(
                    pqk[:, IC : 2 * IC], wkT, xsl, start=True, stop=True
                )
                src = pqk.rearrange("p (s i) -> p s i", s=2)
                if ic % 2:
                    nc.vector.tensor_copy(out=qk8[:, :, sl], in_=src)
                else:
                    nc.scalar.copy(qk8[:, :, sl], src)
                if ic % 2 == 0:
                    pv2[0] = psum_e.tile(
                        [JB, 2 * IC], f32, tag="eps", name=f"pv{ic}"
                    )
                for j4 in range(4):
                    nc.tensor.matmul(
                        pv2[0][:, (4 * (ic % 2) + j4) * C
                               : (4 * (ic % 2) + j4 + 1) * C],
                        xsl[:, j4 * JB : (j4 + 1) * JB],
                        wvT2,
                        start=True,
                        stop=True,
                    )
                if ic % 2 == 1:
                    nc.scalar.copy(
                        vT[:, 2 * ic - 2 : 2 * ic + 2, :, 0:C],
                        pv2[0][:, 0 : 8 * C].rearrange(
                            "p (a b f) -> p a b f", a=4, b=2
                        ),
                    )

            def need_chunks(n):
                while n_chunks[0] < n:
                    emit_chunk(n_chunks[0])
                    n_chunks[0] += 1

            need_chunks(2)
            q8p = [
                qk8[:, 0, 2 * IC * i : 2 * IC * (i + 1)]
                for i in range(N_IC // 2)
            ]
            k8c = qk8[:, 1, :]

            # ---------------- main attention loop ----------------
            # Deferred normalize: pair pr's chain is emitted early in pair
            # pr+1 (Pool + DMA only; nothing the PE waits on).  The last
            # pair's chain runs on DVE after the loop.
            norm_q = []

            def emit_norm(yu, sl2, pr, w):
                # mid-loop normalize: seed on DVE (i16 TS is DVE-only),
                # Newton + big TTs on Pool, partition-broadcast of r via a
                # DRAM round-trip.
                # r0 = +1/s seed: bitcast_bf16(K16 - int16(s_bits))
                r0i = small.tile([C + 1, 2 * IC], i16, tag="r0")
                nc.vector.tensor_scalar(
                    r0i[C : C + 1, 0:w],
                    yu[C : C + 1, 0:w].bitcast(i16),
                    -1.0,
                    float(K16),
                    op0=MUL,
                    op1=ADD,
                )
                r0 = r0i.bitcast(bf16)
                # one Newton step, lands NEGATED: rn = (s*r0 - 2)*r0 = -1/s
                t1 = small.tile([C + 1, 2 * IC], bf16, tag="t1")
                nc.gpsimd.tensor_tensor(
                    out=t1[C : C + 1, 0:w], in0=yu[C : C + 1, 0:w],
                    in1=r0[C : C + 1, 0:w], op=MUL,
                )
                u = small.tile([C + 1, 2 * IC], bf16, tag="u")
                nc.gpsimd.tensor_scalar(
                    u[C : C + 1, 0:w], t1[C : C + 1, 0:w], 1.0, -2.0,
                    op0=MUL, op1=ADD,
                )
                rn = small.tile([C + 1, 2 * IC], bf16, tag="rn")
                nc.gpsimd.tensor_tensor(
                    out=rn[C : C + 1, 0:w], in0=u[C : C + 1, 0:w],
                    in1=r0[C : C + 1, 0:w], op=MUL,
                )
                # broadcast -r over 64 partitions via DRAM round-trip
                nc.sync.dma_start(
                    out=r_d[pr : pr + 1, 0:w], in_=rn[C : C + 1, 0:w]
                )
                rb = small.tile([C, 2 * IC], bf16, tag="rb")
                nc.sync.dma_start(
                    out=rb[:, 0:w],
                    in_=r_d[pr : pr + 1, 0:w].to_broadcast([C, w]),
                )
                # t = yu * (-r);  y = x - t  (f32 residual add)
                t2 = small.tile([C, 2 * IC], bf16, tag="t2")
                nc.gpsimd.tensor_tensor(
                    out=t2[:, 0:w], in0=yu[0:C, 0:w], in1=rb[:, 0:w], op=MUL
                )
                y_sb = ypool.tile([C, 2 * IC], f32)
                nc.gpsimd.tensor_tensor(
                    out=y_sb[:, 0:w], in0=xf32[:, sl2], in1=t2[:, 0:w],
                    op=SUB,
                )
                nc.sync.dma_start(out=y_d[:, sl2], in_=y_sb[:, 0:w])

            def emit_norm_tail(yu, sl2, w):
                # last pair: latency-optimal.  Two independent half-chains;
                # magic seed only, K=1 matmul broadcast; y DMA halves go to
                # both queues (the scalar queue is idle by now).
                for h in range(w // IC):
                    hs = slice(h * IC, (h + 1) * IC)
                    hs2 = slice(sl2.start + h * IC, sl2.start + (h + 1) * IC)
                    r0i = small.tile([C + 1, IC], i16, tag=f"r0t{h}")
                    nc.vector.tensor_scalar(
                        r0i[C : C + 1, :],
                        yu[C : C + 1, hs].bitcast(i16),
                        -1.0,
                        float(K16),
                        op0=MUL,
                        op1=ADD,
                    )
                    r0 = r0i.bitcast(bf16)
                    rb_ps = psum_e.tile([JB, 2 * IC], f32, tag="eps")
                    nc.tensor.matmul(
                        rb_ps[0:C, 0:IC], mones[C : C + 1, :],
                        r0[C : C + 1, :], start=True, stop=True,
                    )
                    t2 = small.tile([C, IC], bf16, tag=f"t2t{h}")
                    nc.vector.tensor_tensor(
                        out=t2, in0=yu[0:C, hs], in1=rb_ps[0:C, 0:IC], op=MUL
                    )
                    y_sb = ypool.tile([C, IC], f32)
                    nc.vector.tensor_tensor(
                        out=y_sb, in0=xf32[:, hs2], in1=t2, op=SUB
                    )
                    eng = nc.scalar if h == 0 else nc.sync
                    eng.dma_start(out=y_d[:, hs2], in_=y_sb)

            # Flat unit-stream over the 4 i-pairs (1024 wide, 32 units
            # (p, ih) each): the next pair's energy matmuls fill the PE
            # wait on the previous pair's tail exps.
            PRS = [
                (0, 2 * IC), (2 * IC * 1, 2 * IC), (2 * IC * 2, 2 * IC),
                (3 * 2 * IC, 2 * IC),
            ]
            UNITS = []
            for pr, (i0, w) in enumerate(PRS):
                if w == 2 * IC:
                    for jb in range(N_JB):
                        UNITS.append((pr, i0, w, jb // 2, jb % 2, jb))
                else:
                    for p in range(NP):
                        UNITS.append((pr, i0, w, p, None, p * 2))
            NT = len(UNITS)
            o_tiles = {}
            a_tiles = {}
            for g in range(NT + PIPE + 1):
                if g < NT:
                    pr, i0, w, p, ih, ju = UNITS[g]
                    if norm_q and g > 0 and UNITS[g - 1][0] != pr:
                        norm_q.pop(0)()
                    if pr == 0:
                        need_chunks(min(N_IC, ju // 4 + 2))
                    e_ps = psum_e.tile([JB, 2 * IC], f32, tag="eps")
                    if ih is not None:
                        # 1024-wide pair: one jb over both i-halves
                        jb = 2 * p + ih
                        kblk = k8c[:, jb * JB : (jb + 1) * JB]
                        qsl = qk8[:, 0, i0 : i0 + w]
                        nc.tensor.matmul(
                            e_ps[:, 0:IC], kblk, qsl[:, 0:IC],
                            start=True, stop=True,
                        )
                        nc.tensor.matmul(
                            e_ps[:, IC : 2 * IC], kblk, qsl[:, IC : 2 * IC],
                            start=True, stop=True,
                        )
                        if ih == 0:
                            a_tiles[(pr, p)] = work.tile(
                                [JB, 2, 2 * IC], fp8e5, tag="aT",
                                name=f"aT{g}",
                            )
                        dst = a_tiles[(pr, p)][:, ih, :]
                    else:
                        # 512-wide pair: both slabs of jb-pair p over one
                        # 512-i window; a-tile laid out [JB, 2, IC]
                        qsl = qk8[:, 0, i0 : i0 + w]
                        for s in range(2):
                            jb = 2 * p + s
                            kblk = k8c[:, jb * JB : (jb + 1) * JB]
                            nc.tensor.matmul(
                                e_ps[:, s * IC : (s + 1) * IC], kblk, qsl,
                                start=True, stop=True,
                            )
                        a_tiles[(pr, p)] = work.tile(
                            [JB, 2, 2 * IC], fp8e5, tag="aT", name=f"aT{g}"
                        )
                        dst = a_tiles[(pr, p)][:, :, 0:IC]
                    if (g * 17) % 32 < 17:
                        nc.scalar.activation(
                            dst, e_ps, EXP, scale=float(1.0 / A5),
                        )
                    else:
                        nc.vector.tensor_scalar(
                            dst.bitcast(i8), e_ps, B5, None, op0=ADD,
                        )
                go = g - PIPE
                if 0 <= go < NT:
                    pro, io0, wo, po, iho, juo = UNITS[go]
                    fire = (iho == 1) or (iho is None)
                    if fire:
                        if po == 0:
                            o_tiles[pro] = psum_o.tile(
                                [2 * C, 2 * IC], f32, tag="op",
                                name=f"op{pro}",
                            )
                        o_ps = o_tiles[pro]
                        aT = a_tiles.pop((pro, po))
                        if iho is not None:
                            nc.tensor.matmul(
                                o_ps[:, 0:IC], vT[:, po], aT[:, :, 0:IC],
                                start=(po == 0), stop=(po == NP - 1),
                                perf_mode=DR,
                            )
                            nc.tensor.matmul(
                                o_ps[:, IC : 2 * IC], vT[:, po],
                                aT[:, :, IC : 2 * IC],
                                start=(po == 0), stop=(po == NP - 1),
                                perf_mode=DR,
                            )
                        else:
                            nc.tensor.matmul(
                                o_ps[:, 0:IC], vT[:, po],
                                aT[:, :, 0:IC],
                                start=(po == 0), stop=(po == NP - 1),
                                perf_mode=DR,
                            )
                        if po == NP - 1:
                            # evacuate rows 0:65 to bf16 on ACT; frees the
                            # psum banks for the next pair's accumulator
                            o_done = o_tiles.pop(pro)
                            sl2 = slice(io0, io0 + wo)
                            yu = small.tile(
                                [C + 1, 2 * IC], bf16, tag="yu"
                            )
                            hw2 = wo // 2
                            nc.scalar.copy(
                                yu[:, 0:hw2], o_done[0 : C + 1, 0:hw2]
                            )
                            nc.vector.tensor_copy(
                                out=yu[:, hw2:wo],
                                in_=o_done[0 : C + 1, hw2:wo],
                            )
                            if pro < len(PRS) - 1:
                                norm_q.append(
                                    lambda yu=yu, sl2=sl2, pro=pro, wo=wo:
                                    emit_norm(yu, sl2, pro, wo)
                                )
                            else:
                                emit_norm_tail(yu, sl2, wo)

            while norm_q:
                norm_q.pop(0)()

    nc.compile()
    return nc


def _get_program():
    if "nc" not in _CACHE:
        _CACHE["nc"] = _build_program()
    return _CACHE["nc"]


def host_pack(inputs):
    """Repack weights/inputs into the device layouts (host-side, cheap).

    Returns (shared, per_batch) where shared holds the weight tensors and
    per_batch is a list of {x, xb} dicts.
    """
    import ml_dtypes

    bf16 = ml_dtypes.bfloat16
    x = np.ascontiguousarray(np.asarray(inputs["x"], dtype=np.float32))
    wq = np.asarray(inputs["wq"], dtype=np.float32)
    bq = np.asarray(inputs["bq"], dtype=np.float32)
    wk = np.asarray(inputs["wk"], dtype=np.float32)
    bk = np.asarray(inputs["bk"], dtype=np.float32)
    wv = np.asarray(inputs["wv"], dtype=np.float32)
    bv = np.asarray(inputs["bv"], dtype=np.float32)
    gamma = float(np.asarray(inputs["gamma"], dtype=np.float32).reshape(()))

    def qk_pack(w, b):
        # [65, 8] = [SQK w^T; SQK b], zero-padded to 128 rows, tiled 16x
        # across the columns -> [128, 128]
        t8 = np.zeros((2 * C, D), dtype=np.float32)
        t8[0:C, :] = SQK * w.T
        t8[C, :] = SQK * b
        return np.ascontiguousarray(np.tile(t8, (1, 16)).astype(bf16))

    wqt = qk_pack(wq, bq)
    wkt = qk_pack(wk, bk)
    wvt = np.zeros((2 * C, C), dtype=np.float32)
    wvt[0:C, :] = gamma * wv.T
    wvt[C, :] = gamma * bv
    wvt = np.ascontiguousarray(wvt.astype(bf16))

    shared = {"wqt": wqt, "wkt": wkt, "wvt": wvt}
    per_batch = []
    for b in range(x.shape[0]):
        xf = np.ascontiguousarray(x[b].reshape(C, N))
        xb = np.zeros((2 * C, N), dtype=bf16)
        xb[0:C, :] = xf.astype(bf16)
        xb[C, :] = bf16(1.0)
        per_batch.append({"x": xf, "xb": np.ascontiguousarray(xb)})
    return shared, per_batch


def kernel(**inputs) -> np.ndarray:
    import time

    nc = _get_program()
    from concourse.bass_utils import run_bass_kernel_spmd

    shared, per_batch = host_pack(inputs)
    in_maps = [{**per_batch[b], **shared} for b in range(B)]
    # the axon-tunneled device occasionally reports a transient
    # NRT_EXEC_UNIT_UNRECOVERABLE; a retry on a fresh execution succeeds
    last_err = None
    for attempt in range(4):
        try:
            res = run_bass_kernel_spmd(nc, in_maps, list(range(B)))
            break
        except Exception as e:  # noqa: BLE001
            last_err = e
            time.sleep(2.0 * (attempt + 1))
    else:
        raise last_err
    out = np.stack(
        [res.results[b]["y"].reshape(C, HH, WW) for b in range(B)], axis=0
    )
    return out.astype(np.float32)


if __name__ == "__main__":
    rng = np.random.default_rng(0)
    inputs = {
        "x": rng.standard_normal((B, C, HH, WW), dtype=np.float32),
        "wq": rng.standard_normal((D, C), dtype=np.float32) * 0.05,
        "bq": rng.standard_normal((D,), dtype=np.float32) * 0.05,
        "wk": rng.standard_normal((D, C), dtype=np.float32) * 0.05,
        "bk": rng.standard_normal((D,), dtype=np.float32) * 0.05,
        "wv": rng.standard_normal((C, C), dtype=np.float32) * 0.05,
        "bv": rng.standard_normal((C,), dtype=np.float32) * 0.05,
        "gamma": rng.standard_normal((1,), dtype=np.float32),
    }
    out = kernel(**inputs)
    print("out", out.shape, out.dtype, float(np.abs(out).max()))


# revision 39
# speedup vs baseline: 1.0336x; 1.0336x over previous
"""Trainium2 Bass kernel for nn_AttentionLayer (sparse_attention).

Reference computation (per batch b):
    q = wq @ x + bq          [8, N]     (1x1 conv, d=8, N=H*W=4096)
    k = wk @ x + bk          [8, N]
    v = wv @ x + bv          [64, N]
    energy = q^T k           [N, N]
    attn = softmax(energy, axis=-1)
    out = gamma * (v @ attn^T) + x
Sharding: data-parallel over batch; one batch element per NeuronCore.

Device-side work (the measured NEFF): q/k/v projections, the N x N
energy matmuls, softmax, the output matmuls, normalize, residual.
Host-side (kernel(), unmeasured like any input sharding): weight
REPACKING only - transposes, SQK/gamma/bias folding, 16x replication,
bf16/f32 dtype staging of x.  No model matmuls happen on host.

Architecture (hardware-measured rates drove every choice):
  - PE psum write port = 128 partitions x 1 column/cycle @ 2.4 GHz is
    the matmul wall: energy emits N^2/128 = 131k columns (55 us), the
    out accumulation 16 slab-passes x 4096 i / 512-per-bank = 65k
    columns (27 us).  Row-tiled / partial-K matmuls share the same port
    (measured) AND de-assert the HAM activity monitor (PE drops to 1.2
    GHz), so energy matmuls stay plain K=128 fp8 (16 replicas of the
    d=8 q/k, SQK-scaled so psum = A5 * q.k exactly).
  - Out (v @ attn^T): DoubleRow fp8: lhsT = vT jb-pair [128, 2, 128],
    rhs = aT [128, 2, 512]; vT col 64 = ones accumulates the softmax
    denominator in psum row 64.
  - exp: split ACT (true exp -> fp8e5, (FD+352)/1.2 ns) and DVE
    (Schraudolph e5m2 bits: i8(round(psum + B5)), (FD+120)/0.96 ns) -
    the ONLY two engines that read PSUM; their combined stream rate
    (2.16 elem/ns) is the softmax floor (~61 us for 128k FD).
  - normalize on Pool (gpsimd, idle otherwise): 1/s via bf16 magic
    seed + one Newton step; r broadcast over partitions via a DRAM
    round-trip; y = x - yu*(-r) with the residual add in f32.
  - prep: every DMA costs ~600ns of queue time; x lands via both HWDGE
    queues; chunk emission interleaves with the first i-pair's units so
    the attention loop starts as soon as chunks 0-1 are evacuated.

Accuracy: fp8 q/k/v + e5m2 attn weights + bf16 normalize cost ~3e-3
final relative error (tolerance 2e-2).
"""

import os
import sys

import numpy as np

sys.path.insert(0, "/opt/trn_rl_repo")

B, C, HH, WW = 8, 64, 64, 64
N = HH * WW  # 4096
D = 8  # qk channels
IC = 512  # i-chunk
N_IC = N // IC  # 8
JB = 128  # j-block
N_JB = N // JB  # 32
NP = N_JB // 2  # 16 jb-pairs

A5 = float(4.0 / np.log(2.0))
B5 = float(4 * 15.0 - 0.5)
SQK = float(0.25 * np.sqrt(A5))  # per-side scale: 16 reps * SQK^2 = A5
K16 = 0x7EF0  # bf16 reciprocal magic
PIPE = 4

# jb's whose exp runs on ACT (17, evenly spread); rest on DVE (15)
ACT_SET = frozenset(j for j in range(32) if (j * 17) // 32 != ((j + 1) * 17) // 32)

_CACHE = {}


def _build_program():
    import concourse.bass as bass
    import concourse.tile as tile
    from concourse import bacc, mybir

    f32 = mybir.dt.float32
    bf16 = mybir.dt.bfloat16
    i8 = mybir.dt.int8
    i16 = mybir.dt.int16
    fp8e4 = mybir.dt.float8e4
    fp8e5 = mybir.dt.float8e5
    EXP = mybir.ActivationFunctionType.Exp
    DR = mybir.MatmulPerfMode.DoubleRow
    MUL = mybir.AluOpType.mult
    ADD = mybir.AluOpType.add
    SUB = mybir.AluOpType.subtract

    nc = bacc.Bacc(
        "TRN2", target_bir_lowering=False, debug=False, enable_asserts=False
    )

    # Host-prepacked inputs (see _host_pack): xb = [x; ones; zeros] bf16,
    # wqt/wkt = replicated SQK-scaled [wq;bq]^T bf16 [128, 128],
    # wvt = [gamma wv^T; gamma bv; zeros] bf16 [128, 64].
    x_d = nc.dram_tensor("x", [C, N], f32, kind="ExternalInput").ap()
    xb_d = nc.dram_tensor("xb", [2 * C, N], bf16, kind="ExternalInput").ap()
    wqt_d = nc.dram_tensor("wqt", [2 * C, 2 * C], bf16, kind="ExternalInput").ap()
    wkt_d = nc.dram_tensor("wkt", [2 * C, 2 * C], bf16, kind="ExternalInput").ap()
    wvt_d = nc.dram_tensor("wvt", [2 * C, C], bf16, kind="ExternalInput").ap()
    y_d = nc.dram_tensor("y", [C, N], f32, kind="ExternalOutput").ap()
    r_d = nc.dram_tensor("r_scr", [5, 2 * IC], bf16, kind="Internal").ap()

    with tile.TileContext(nc) as tc:
        from contextlib import ExitStack

        with ExitStack() as ctx:
            consts = ctx.enter_context(tc.tile_pool(name="consts", bufs=1))
            bigs = ctx.enter_context(tc.tile_pool(name="bigs", bufs=1))
            work = ctx.enter_context(tc.tile_pool(name="work", bufs=4))
            ypool = ctx.enter_context(tc.tile_pool(name="ypool", bufs=2))
            small = ctx.enter_context(tc.tile_pool(name="small", bufs=4))

            # ---------------- DMAs ----------------
            # Weights first (tiny), then xb in 3 pieces split across both
            # HWDGE queues so chunk 0 lands earliest, then xf32 (only
            # needed for the residual from ~35us on).
            wqT = consts.tile([2 * C, 2 * C], bf16)
            wkT = consts.tile([2 * C, 2 * C], bf16)
            wvT2 = consts.tile([2 * C, C], bf16)
            x2c = bigs.tile([2 * C, N], bf16)
            xf32 = bigs.tile([C, N], f32)
            nc.scalar.dma_start(out=x2c[:, 0:IC], in_=xb_d[:, 0:IC])
            nc.scalar.dma_start(out=x2c[:, IC : 2 * IC], in_=xb_d[:, IC : 2 * IC])
            nc.scalar.dma_start(
                out=x2c[:, 2 * IC : 5 * IC], in_=xb_d[:, 2 * IC : 5 * IC]
            )
            nc.sync.dma_start(out=wqT, in_=wqt_d)
            nc.sync.dma_start(out=wkT, in_=wkt_d)
            nc.sync.dma_start(out=wvT2, in_=wvt_d)
            nc.sync.dma_start(out=x2c[:, 5 * IC :], in_=xb_d[:, 5 * IC :])
            nc.sync.dma_start(out=xf32[:, 0 : N // 2], in_=x_d[:, 0 : N // 2])
            nc.sync.dma_start(out=xf32[:, N // 2 :], in_=x_d[:, N // 2 :])

            # warm-up stationary: depends only on a DVE memset
            wconst = consts.tile([C, C], bf16)
            nc.vector.memset(wconst.bitcast(f32), 0.0)
            mones = consts.tile([65, C], bf16)
            nc.vector.memset(mones, -1.0)

            # vT: [128, NP, 2, 128] fp8e4; [j, p, s, 0:64] = gamma*v^T for
            # j-block 2p+s, col 64 = ones (denominator), cols 65:127 = 0.
            # On GPSIMD (idle in prep; no longer fights make_identity).
            vT = bigs.tile([JB, NP, 2, JB], fp8e4)
            nc.gpsimd.memset(vT[:, :, :, C : C + 1], 1.0)
            nc.gpsimd.memset(vT[:, :, :, C + 1 :], 0.0)

            # warm the Exp activation table early (table load ~1.3us)
            warm = consts.tile([1, 8], f32)
            nc.scalar.activation(warm, wconst[0:1, 0:8], EXP)

            # Unified PSUM rotation: warmers, prep projections and loop
            # energy tiles share ONE bufs=3 pool of [128, 1024] f32 tiles
            # (2 banks) -- no pool-close barrier between warm-up and the
            # first projection, and the loop's first energy matmuls chase
            # the prep evacuations through the same rotation.
            psum_e = ctx.enter_context(
                tc.tile_pool(name="psum_e", bufs=3, space="PSUM")
            )
            psum_o = ctx.enter_context(
                tc.tile_pool(name="psum_o", bufs=1, space="PSUM")
            )

            # ramp-warmer: PE busy from ~6.6us so the HAM clock grant hits
            # full speed before/while the projections run.
            wsc = psum_e.tile([JB, 2 * IC], f32, tag="eps", name="wsc")
            for _ in range(8):
                nc.tensor.matmul(
                    wsc[0:C, 0:C], wconst, wconst, start=True, stop=True
                )

            # ---------------- projections ----------------
            # qk8 [128, 2, N] fp8e4: slab 0 = q, slab 1 = k (16 partition-
            # replicas each).  q and k land in ONE psum tile per chunk so a
            # single FD-1024 evacuation copy moves both; copies alternate
            # ACT/DVE.  v psums pair 2 chunks -> one FD-512 ACT copy.
            # Chunk emission interleaves with the first i-pair's units
            # (need_chunks) so the attention loop starts once chunks 0-1
            # are evacuated.
            # bridge warmers: keep the PE busy between the early warm block
            # and the first projection (gated by the x DMA landing ~11.5us)
            # so the HAM activity window never sees an idle gap.  They live
            # in psum_o's banks, whose first real use is much later.
            brg = psum_o.tile([2 * C, 2 * IC], f32, tag="op", name="brg")
            for _ in range(13):
                nc.tensor.matmul(
                    brg[0:C, 0:C], wconst, wconst, start=True, stop=True
                )

            qk8 = bigs.tile([2 * C, 2, N], fp8e4)
            pv2 = [None]
            n_chunks = [0]

            def emit_chunk(ic):
                sl = slice(ic * IC, (ic + 1) * IC)
                xsl = x2c[:, sl]
                pqk = psum_e.tile([2 * C, 2 * IC], f32, tag="eps")
                nc.tensor.matmul(
                    pqk[:, 0:IC], wqT, xsl, start=True, stop=True
                )
                nc.tensor.matmul(
                    pqk[:, IC : 2 * IC], wkT, xsl, start=True, stop=True
                )
                src = pqk.rearrange("p (s i) -> p s i", s=2)
                if ic % 2:
                    nc.vector.tensor_copy(out=qk8[:, :, sl], in_=src)
                else:
                    nc.scalar.copy(qk8[:, :, sl], src)
                if ic % 2 == 0:
                    pv2[0] = psum_e.tile(
                        [JB, 2 * IC], f32, tag="eps", name=f"pv{ic}"
                    )
                for j4 in range(4):
                    nc.tensor.matmul(
                        pv2[0][:, (4 * (ic % 2) + j4) * C
                               : (4 * (ic % 2) + j4 + 1) * C],
                        xsl[:, j4 * JB : (j4 + 1) * JB],
                        wvT2,
                        start=True,
                        stop=True,
                    )
                if ic % 2 == 1:
                    nc.scalar.copy(
                        vT[:, 2 * ic - 2 : 2 * ic + 2, :, 0:C],
                        pv2[0][:, 0 : 8 * C].rearrange(
                            "p (a b f) -> p a b f", a=4, b=2
                        ),
                    )

            def need_chunks(n):
                while n_chunks[0] < n:
                    emit_chunk(n_chunks[0])
                    n_chunks[0] += 1

            need_chunks(2)
            q8p = [
                qk8[:, 0, 2 * IC * i : 2 * IC * (i + 1)]
                for i in range(N_IC // 2)
            ]
            k8c = qk8[:, 1, :]

            # ---------------- main attention loop ----------------
            # Deferred normalize: pair pr's chain is emitted early in pair
            # pr+1 (Pool + DMA only; nothing the PE waits on).  The last
            # pair's chain runs on DVE after the loop.
            norm_q = []

            def emit_norm(yu, sl2, pr, w):
                # mid-loop normalize: seed on DVE (i16 TS is DVE-only),
                # Newton + big TTs on Pool, partition-broadcast of r via a
                # DRAM round-trip.
                # r0 = +1/s seed: bitcast_bf16(K16 - int16(s_bits))
                r0i = small.tile([C + 1, 2 * IC], i16, tag="r0")
                nc.vector.tensor_scalar(
                    r0i[C : C + 1, 0:w],
                    yu[C : C + 1, 0:w].bitcast(i16),
                    -1.0,
                    float(K16),
                    op0=MUL,
                    op1=ADD,
                )
                r0 = r0i.bitcast(bf16)
                # one Newton step, lands NEGATED: rn = (s*r0 - 2)*r0 = -1/s
                t1 = small.tile([C + 1, 2 * IC], bf16, tag="t1")
                nc.gpsimd.tensor_tensor(
                    out=t1[C : C + 1, 0:w], in0=yu[C : C + 1, 0:w],
                    in1=r0[C : C + 1, 0:w], op=MUL,
                )
                u = small.tile([C + 1, 2 * IC], bf16, tag="u")
                nc.gpsimd.tensor_scalar(
                    u[C : C + 1, 0:w], t1[C : C + 1, 0:w], 1.0, -2.0,
                    op0=MUL, op1=ADD,
                )
                rn = small.tile([C + 1, 2 * IC], bf16, tag="rn")
                nc.gpsimd.tensor_tensor(
                    out=rn[C : C + 1, 0:w], in0=u[C : C + 1, 0:w],
                    in1=r0[C : C + 1, 0:w], op=MUL,
                )
                # broadcast -r over 64 partitions via DRAM round-trip
                nc.sync.dma_start(
                    out=r_d[pr : pr + 1, 0:w], in_=rn[C : C + 1, 0:w]
                )
                rb = small.tile([C, 2 * IC], bf16, tag="rb")
                nc.sync.dma_start(
                    out=rb[:, 0:w],
                    in_=r_d[pr : pr + 1, 0:w].to_broadcast([C, w]),
                )
                # t = yu * (-r);  y = x - t  (f32 residual add)
                t2 = small.tile([C, 2 * IC], bf16, tag="t2")
                nc.gpsimd.tensor_tensor(
                    out=t2[:, 0:w], in0=yu[0:C, 0:w], in1=rb[:, 0:w], op=MUL
                )
                y_sb = ypool.tile([C, 2 * IC], f32)
                nc.gpsimd.tensor_tensor(
                    out=y_sb[:, 0:w], in0=xf32[:, sl2], in1=t2[:, 0:w],
                    op=SUB,
                )
                nc.sync.dma_start(out=y_d[:, sl2], in_=y_sb[:, 0:w])

            def emit_norm_tail(yu, sl2, w):
                # last pair: latency-optimal.  Two independent half-chains;
                # magic seed only, K=1 matmul broadcast; y DMA halves go to
                # both queues (the scalar queue is idle by now).
                for h in range(w // IC):
                    hs = slice(h * IC, (h + 1) * IC)
                    hs2 = slice(sl2.start + h * IC, sl2.start + (h + 1) * IC)
                    r0i = small.tile([C + 1, IC], i16, tag=f"r0t{h}")
                    nc.vector.tensor_scalar(
                        r0i[C : C + 1, :],
                        yu[C : C + 1, hs].bitcast(i16),
                        -1.0,
                        float(K16),
                        op0=MUL,
                        op1=ADD,
                    )
                    r0 = r0i.bitcast(bf16)
                    rb_ps = psum_e.tile([JB, 2 * IC], f32, tag="eps")
                    nc.tensor.matmul(
                        rb_ps[0:C, 0:IC], mones[C : C + 1, :],
                        r0[C : C + 1, :], start=True, stop=True,
                    )
                    t2 = small.tile([C, IC], bf16, tag=f"t2t{h}")
                    nc.vector.tensor_tensor(
                        out=t2, in0=yu[0:C, hs], in1=rb_ps[0:C, 0:IC], op=MUL
                    )
                    y_sb = ypool.tile([C, IC], f32)
                    nc.vector.tensor_tensor(
                        out=y_sb, in0=xf32[:, hs2], in1=t2, op=SUB
                    )
                    eng = nc.scalar if h == 0 else nc.sync
                    eng.dma_start(out=y_d[:, hs2], in_=y_sb)

            # Flat unit-stream over the 4 i-pairs (1024 wide, 32 units
            # (p, ih) each): the next pair's energy matmuls fill the PE
            # wait on the previous pair's tail exps.
            PRS = [
                (0, 2 * IC), (2 * IC * 1, 2 * IC), (2 * IC * 2, 2 * IC),
                (3 * 2 * IC, 2 * IC),
            ]
            UNITS = []
            for pr, (i0, w) in enumerate(PRS):
                if w == 2 * IC:
                    for jb in range(N_JB):
                        UNITS.append((pr, i0, w, jb // 2, jb % 2, jb))
                else:
                    for p in range(NP):
                        UNITS.append((pr, i0, w, p, None, p * 2))
            NT = len(UNITS)
            o_tiles = {}
            a_tiles = {}
            for g in range(NT + PIPE + 1):
                if g < NT:
                    pr, i0, w, p, ih, ju = UNITS[g]
                    if norm_q and g % 8 == 4:
                        norm_q.pop(0)()
                    if pr == 0:
                        need_chunks(min(N_IC, ju // 4 + 2))
                    e_ps = psum_e.tile([JB, 2 * IC], f32, tag="eps")
                    if ih is not None:
                        # 1024-wide pair: one jb over both i-halves
                        jb = 2 * p + ih
                        kblk = k8c[:, jb * JB : (jb + 1) * JB]
                        qsl = qk8[:, 0, i0 : i0 + w]
                        nc.tensor.matmul(
                            e_ps[:, 0:IC], kblk, qsl[:, 0:IC],
                            start=True, stop=True,
                        )
                        nc.tensor.matmul(
                            e_ps[:, IC : 2 * IC], kblk, qsl[:, IC : 2 * IC],
                            start=True, stop=True,
                        )
                        if ih == 0:
                            a_tiles[(pr, p)] = work.tile(
                                [JB, 2, 2 * IC], fp8e5, tag="aT",
                                name=f"aT{g}",
                            )
                        dst = a_tiles[(pr, p)][:, ih, :]
                    else:
                        # 512-wide pair: both slabs of jb-pair p over one
                        # 512-i window; a-tile laid out [JB, 2, IC]
                        qsl = qk8[:, 0, i0 : i0 + w]
                        for s in range(2):
                            jb = 2 * p + s
                            kblk = k8c[:, jb * JB : (jb + 1) * JB]
                            nc.tensor.matmul(
                                e_ps[:, s * IC : (s + 1) * IC], kblk, qsl,
                                start=True, stop=True,
                            )
                        a_tiles[(pr, p)] = work.tile(
                            [JB, 2, 2 * IC], fp8e5, tag="aT", name=f"aT{g}"
                        )
                        dst = a_tiles[(pr, p)][:, :, 0:IC]
                    if (g * 17) % 32 < 17:
                        nc.scalar.activation(
                            dst, e_ps, EXP, scale=float(1.0 / A5),
                        )
                    else:
                        nc.vector.tensor_scalar(
                            dst.bitcast(i8), e_ps, B5, None, op0=ADD,
                        )
                go = g - PIPE
                if 0 <= go < NT:
                    pro, io0, wo, po, iho, juo = UNITS[go]
                    fire = (iho == 1) or (iho is None)
                    if fire:
                        if po == 0:
                            o_tiles[pro] = psum_o.tile(
                                [2 * C, 2 * IC], f32, tag="op",
                                name=f"op{pro}",
                            )
                        o_ps = o_tiles[pro]
                        aT = a_tiles.pop((pro, po))
                        if iho is not None:
                            nc.tensor.matmul(
                                o_ps[:, 0:IC], vT[:, po], aT[:, :, 0:IC],
                                start=(po == 0), stop=(po == NP - 1),
                                perf_mode=DR,
                            )
                            nc.tensor.matmul(
                                o_ps[:, IC : 2 * IC], vT[:, po],
                                aT[:, :, IC : 2 * IC],
                                start=(po == 0), stop=(po == NP - 1),
                                perf_mode=DR,
                            )
                        else:
                            nc.tensor.matmul(
                                o_ps[:, 0:IC], vT[:, po],
                                aT[:, :, 0:IC],
                                start=(po == 0), stop=(po == NP - 1),
                                perf_mode=DR,
                            )
                        if po == NP - 1:
                            # evacuate rows 0:65 to bf16 on ACT; frees the
                            # psum banks for the next pair's accumulator
                            o_done = o_tiles.pop(pro)
                            sl2 = slice(io0, io0 + wo)
                            yu = small.tile(
                                [C + 1, 2 * IC], bf16, tag="yu"
                            )
                            hw2 = wo // 2
                            nc.scalar.copy(
                                yu[:, 0:hw2], o_done[0 : C + 1, 0:hw2]
                            )
                            nc.vector.tensor_copy(
                                out=yu[:, hw2:wo],
                                in_=o_done[0 : C + 1, hw2:wo],
                            )
                            if pro < len(PRS) - 1:
                                norm_q.append(
                                    lambda yu=yu, sl2=sl2, pro=pro, wo=wo:
                                    emit_norm(yu, sl2, pro, wo)
                                )
                            else:
                                emit_norm_tail(yu, sl2, wo)

            while norm_q:
                norm_q.pop(0)()

    nc.compile()
    return nc


def _get_program():
    if "nc" not in _CACHE:
        _CACHE["nc"] = _build_program()
    return _CACHE["nc"]


def host_pack(inputs):
    """Repack weights/inputs into the device layouts (host-side, cheap).

    Returns (shared, per_batch) where shared holds the weight tensors and
    per_batch is a list of {x, xb} dicts.
    """
    import ml_dtypes

    bf16 = ml_dtypes.bfloat16
    x = np.ascontiguousarray(np.asarray(inputs["x"], dtype=np.float32))
    wq = np.asarray(inputs["wq"], dtype=np.float32)
    bq = np.asarray(inputs["bq"], dtype=np.float32)
    wk = np.asarray(inputs["wk"], dtype=np.float32)
    bk = np.asarray(inputs["bk"], dtype=np.float32)
    wv = np.asarray(inputs["wv"], dtype=np.float32)
    bv = np.asarray(inputs["bv"], dtype=np.float32)
    gamma = float(np.asarray(inputs["gamma"], dtype=np.float32).reshape(()))

    def qk_pack(w, b):
        # [65, 8] = [SQK w^T; SQK b], zero-padded to 128 rows, tiled 16x
        # across the columns -> [128, 128]
        t8 = np.zeros((2 * C, D), dtype=np.float32)
        t8[0:C, :] = SQK * w.T
        t8[C, :] = SQK * b
        return np.ascontiguousarray(np.tile(t8, (1, 16)).astype(bf16))

    wqt = qk_pack(wq, bq)
    wkt = qk_pack(wk, bk)
    wvt = np.zeros((2 * C, C), dtype=np.float32)
    wvt[0:C, :] = gamma * wv.T
    wvt[C, :] = gamma * bv
    wvt = np.ascontiguousarray(wvt.astype(bf16))

    shared = {"wqt": wqt, "wkt": wkt, "wvt": wvt}
    per_batch = []
    for b in range(x.shape[0]):
        xf = np.ascontiguousarray(x[b].reshape(C, N))
        xb = np.zeros((2 * C, N), dtype=bf16)
        xb[0:C, :] = xf.astype(bf16)
        xb[C, :] = bf16(1.0)
        per_batch.append({"x": xf, "xb": np.ascontiguousarray(xb)})
    return shared, per_batch


def kernel(**inputs) -> np.ndarray:
    import time

    nc = _get_program()
    from concourse.bass_utils import run_bass_kernel_spmd

    shared, per_batch = host_pack(inputs)
    in_maps = [{**per_batch[b], **shared} for b in range(B)]
    # the axon-tunneled device occasionally reports a transient
    # NRT_EXEC_UNIT_UNRECOVERABLE; a retry on a fresh execution succeeds
    last_err = None
    for attempt in range(4):
        try:
            res = run_bass_kernel_spmd(nc, in_maps, list(range(B)))
            break
        except Exception as e:  # noqa: BLE001
            last_err = e
            time.sleep(2.0 * (attempt + 1))
    else:
        raise last_err
    out = np.stack(
        [res.results[b]["y"].reshape(C, HH, WW) for b in range(B)], axis=0
    )
    return out.astype(np.float32)


if __name__ == "__main__":
    rng = np.random.default_rng(0)
    inputs = {
        "x": rng.standard_normal((B, C, HH, WW), dtype=np.float32),
        "wq": rng.standard_normal((D, C), dtype=np.float32) * 0.05,
        "bq": rng.standard_normal((D,), dtype=np.float32) * 0.05,
        "wk": rng.standard_normal((D, C), dtype=np.float32) * 0.05,
        "bk": rng.standard_normal((D,), dtype=np.float32) * 0.05,
        "wv": rng.standard_normal((C, C), dtype=np.float32) * 0.05,
        "bv": rng.standard_normal((C,), dtype=np.float32) * 0.05,
        "gamma": rng.standard_normal((1,), dtype=np.float32),
    }
    out = kernel(**inputs)
    print("out", out.shape, out.dtype, float(np.abs(out).max()))
